# revision 3
# baseline (speedup 1.0000x reference)
"""AdaptiveCLPL loss on 8 TRN2 NeuronCores (Bass/Tile) — v2.

loss = mean_b [ psi(avg_cand) + sum_head psi(-l)*(1-mask) + ts*sum_samp psi(-l)*(1-is_cand) ]
with psi(u) = softplus(-u) = Ln(Exp(-u)+1) (no native softplus table).

Decomposition (only term1 is per-row nonlinear; everything else sums):
  total = sum_b softplus(-avg_b)
        + [sum_{head block} softplus(l)    - sum_k uniq*inhead*softplus(l_cand)]
        + ts*[sum_{sampled rows} softplus(l) - sum_k uniq*mult*softplus(l_cand)]

Per-core layout: transposed batch shard lT = logits[rows_perm].T ([C, RB]
row-major); every lT row is a 1KB chunk addressed by class. Candidate values
come from dma_gather (one 1KB descriptor per candidate). Key tricks vs v1:
  - overlapping int16 windows [0,32768) and [C-32768, C): candidates in the
    overlap go to either window, so every partition holds EXACTLY nj0+nj1
    slots -> zero descriptor padding (2560 descriptors, the floor).
  - sampled rows ride the same gather (trailing -1 idxs are skipped by the
    ucode), replacing the slow gpsimd indirect DMA.
  - a 16-idx dummy gather issued first pays the gpsimd 'mlp' library IRAM
    load while the idx/aux DMAs are still in flight.
  - gathers split into sub-calls with separate dest tiles so descriptor
    generation, wire transfer and vector extraction pipeline.
  - rows are packed 2 per partition; the shard column of row (p,g) is 2p+g.
"""

import numpy as np

B, C, K = 2048, 50000, 10
HEAD, S = 2000, 100
TSCALE = float(C - HEAD) / float(S)  # 480.0
NCORES = 8
RB = B // NCORES  # 256 rows per core
P = 128
HP = 125          # head tile partitions; 2000 rows = 125 * 16
HB = HEAD // HP   # 16 blocks -> 16KB contiguous per partition
ES = 256          # chunk = one lT row (1KB)
WIN = 32768
LO1 = C - WIN     # 17232; window1 = [LO1, C)
GMAX = 2          # exactly 2 rows per partition
NSUB = 2          # sub-gathers per window

_CACHE = {}


def _pack_rows(h0, h1, nj_target, rng):
    """Pair 2*P rows into P partitions s.t. per-partition hard-window counts
    stay <= nj_target. Returns part[r] in [0,P)."""
    nrows = len(h0)
    order = np.argsort(-h0, kind="stable")
    part = np.zeros(nrows, np.int64)
    for i in range(P):
        part[order[i]] = i
        part[order[nrows - 1 - i]] = i
    H0 = np.bincount(part, weights=h0, minlength=P)
    H1 = np.bincount(part, weights=h1, minlength=P)

    def viol(a0, a1):
        return max(a0 - nj_target, 0) + max(a1 - nj_target, 0)

    cur = sum(viol(H0[p], H1[p]) for p in range(P))
    it = 0
    while cur > 0 and it < 20000:
        it += 1
        a, b = rng.integers(0, nrows, 2)
        pa, pb = part[a], part[b]
        if pa == pb:
            continue
        old = viol(H0[pa], H1[pa]) + viol(H0[pb], H1[pb])
        H0[pa] += h0[b] - h0[a]; H1[pa] += h1[b] - h1[a]
        H0[pb] += h0[a] - h0[b]; H1[pb] += h1[a] - h1[b]
        new = viol(H0[pa], H1[pa]) + viol(H0[pb], H1[pb])
        if new <= old:
            part[a], part[b] = pb, pa
            cur += new - old
        else:
            H0[pa] -= h0[b] - h0[a]; H1[pa] -= h1[b] - h1[a]
            H0[pb] -= h0[a] - h0[b]; H1[pb] -= h1[a] - h1[b]
    return part, cur == 0


def prep_inputs(logits, candidates, sampled_indices):
    """Full inputs -> (in_maps, meta). Host work is sharding + index math only."""
    logits = np.asarray(logits)
    candidates = np.asarray(candidates)
    sampled_indices = np.asarray(sampled_indices)
    assert logits.shape == (B, C) and candidates.shape == (B, K)
    srow = (HEAD + sampled_indices.astype(np.int64)).astype(np.int64)  # [S]
    svals, scounts = np.unique(srow, return_counts=True)
    smult = dict(zip(svals.tolist(), scounts.tolist()))

    # sampled rows -> windows (balance the flexible ones)
    s_w = np.where(srow < LO1, 0, np.where(srow >= WIN, 1, -1))
    flex = np.where(s_w < 0)[0]
    n0 = int((s_w == 0).sum())
    n1 = int((s_w == 1).sum())
    for j in flex:
        if n0 <= n1:
            s_w[j] = 0; n0 += 1
        else:
            s_w[j] = 1; n1 += 1
    ns0, ns1 = n0, n1
    sidx_w = [srow[s_w == 0] - 0, srow[s_w == 1] - LO1]

    rng = np.random.default_rng(12345)
    cores = []
    nj_need = [1, 1]
    for i in range(NCORES):
        rows = slice(i * RB, (i + 1) * RB)
        cand = candidates[rows].astype(np.int64)          # [RB, K]
        valid = cand >= 0
        uniq = valid.copy()
        for k in range(1, K):
            dup = (cand[:, :k] == cand[:, k:k + 1]).any(axis=1)
            uniq[:, k] &= ~dup
        uniqf = uniq.astype(np.float32)
        cnt = np.maximum((uniq & valid).sum(axis=1), 1).astype(np.float32)
        inhead = (cand < HEAD).astype(np.float32)
        mult = np.vectorize(lambda c: smult.get(int(c), 0))(cand).astype(np.float32)
        wcorr_rk = -uniqf * (inhead + TSCALE * mult)      # [RB, K]

        h0 = (valid & (cand < LO1)).sum(axis=1)
        h1 = (valid & (cand >= WIN)).sum(axis=1)
        tot = valid.sum(axis=1)
        part, ok = _pack_rows(h0.astype(np.int64), h1.astype(np.int64), K, rng)
        # per-partition group = order of appearance
        grp = np.zeros(RB, np.int64)
        seen = {}
        for r in range(RB):
            p = int(part[r])
            grp[r] = seen.get(p, 0)
            seen[p] = grp[r] + 1
        assert max(seen.values()) <= GMAX

        # window assignment per candidate
        cw = np.full((RB, K), -1, np.int64)
        cw[valid & (cand < LO1)] = 0
        cw[valid & (cand >= WIN)] = 1
        H0 = np.bincount(part, weights=(cw == 0).sum(1), minlength=P).astype(np.int64)
        # flexible candidates per partition: fill window0 up to K slots
        for r in range(RB):
            p = int(part[r])
            for k in range(K):
                if valid[r, k] and cw[r, k] < 0:
                    if H0[p] < K:
                        cw[r, k] = 0; H0[p] += 1
                    else:
                        cw[r, k] = 1
        W0c = np.bincount(part, weights=(cw == 0).sum(1), minlength=P).astype(np.int64)
        W1c = np.bincount(part, weights=(cw == 1).sum(1), minlength=P).astype(np.int64)
        nj_need[0] = max(nj_need[0], int(W0c.max()))
        nj_need[1] = max(nj_need[1], int(W1c.max()))
        cores.append((cand, valid, uniqf, cnt, wcorr_rk, part, grp, cw))

    nj0, nj1 = nj_need
    njtot = nj0 + nj1
    meta = (nj0, nj1, ns0, ns1)

    # idx16 column layout: per window, NSUB sub-calls; then 2 sampled calls
    def subslots(nj):
        base = nj // NSUB
        sizes = [base + (1 if s < nj % NSUB else 0) for s in range(NSUB)]
        return [s for s in sizes if s > 0]

    sub0, sub1 = subslots(nj0), subslots(nj1)
    ncols_c = (nj0 + nj1) * P // 16
    ncols_s = (128 // 16) * 2
    AUXW = njtot + njtot + GMAX * njtot + GMAX + ES

    in_maps = []
    for i in range(NCORES):
        cand, valid, uniqf, cnt, wcorr_rk, part, grp, cw = cores[i]
        rows = slice(i * RB, (i + 1) * RB)
        # shard column of row r = 2*part[r] + grp[r]
        col = (2 * part + grp).astype(np.int64)
        perm = np.zeros(RB, np.int64)
        perm[col] = np.arange(RB)
        lT = np.ascontiguousarray(logits[rows][perm].T.astype(np.float32, copy=False))

        # slot tables
        idxs = [np.zeros(nj0 * P, np.int16), np.zeros(nj1 * P, np.int16)]
        offt = np.full((P, njtot), -1.0, np.float32)
        wcorr = np.zeros((P, njtot), np.float32)
        wg = np.zeros((P, GMAX * njtot), np.float32)
        fill = np.zeros((P, 2), np.int64)
        base_j = [0, nj0]
        base_lo = [0, LO1]
        for r in range(RB):
            p, g = int(part[r]), int(grp[r])
            for k in range(K):
                if not valid[r, k]:
                    continue
                w = int(cw[r, k])
                j = int(fill[p, w]); fill[p, w] += 1
                idxs[w][j * P + p] = cand[r, k] - base_lo[w]
                jj = base_j[w] + j
                offt[p, jj] = float(col[r])
                wcorr[p, jj] = wcorr_rk[r, k]
                wg[p, g * njtot + jj] = uniqf[r, k]
        rcnt = np.zeros((P, GMAX), np.float32)
        rcnt[part, grp] = 1.0 / cnt
        iota = np.broadcast_to(np.arange(ES, dtype=np.float32), (P, ES)).copy()

        # wrap idx lists: per sub-call, [n/16, 16].T tiled to 128 partitions
        cols = []
        for w, subs in ((0, sub0), (1, sub1)):
            j0 = 0
            for snj in subs:
                flat = idxs[w][j0 * P:(j0 + snj) * P]
                wrapped = flat.reshape(snj * P // 16, 16).T
                cols.append(np.tile(wrapped, (8, 1)))
                j0 += snj
        for w in (0, 1):
            flat = np.full(128, -1, np.int16)
            flat[:len(sidx_w[w])] = sidx_w[w].astype(np.int16)
            wrapped = flat.reshape(8, 16).T
            cols.append(np.tile(wrapped, (8, 1)))
        idx16 = np.ascontiguousarray(np.concatenate(cols, axis=1))
        assert idx16.shape == (P, ncols_c + ncols_s)

        auxcat = np.ascontiguousarray(np.concatenate(
            [offt, wcorr, wg, rcnt, iota], axis=1))
        assert auxcat.shape == (P, AUXW)
        in_maps.append({"lT": lT, "idx16": idx16, "aux": auxcat})
    return in_maps, meta


def _build(meta, enable_asserts=False):
    import concourse.bass as bass
    import concourse.tile as tile
    from concourse import bacc, bass_isa, mybir

    nj0, nj1, ns0, ns1 = meta
    njtot = nj0 + nj1

    f32 = mybir.dt.float32
    i16 = mybir.dt.int16
    AF = mybir.ActivationFunctionType
    OP = mybir.AluOpType
    AX = mybir.AxisListType

    nc = bacc.Bacc(
        "TRN2",
        target_bir_lowering=False,
        debug=False,
        enable_asserts=enable_asserts,
        num_devices=NCORES,
    )

    def subslots(nj):
        base = nj // NSUB
        sizes = [base + (1 if s < nj % NSUB else 0) for s in range(NSUB)]
        return [s for s in sizes if s > 0]

    sub0, sub1 = subslots(nj0), subslots(nj1)
    ncols_c = njtot * P // 16
    ncols_s = (128 // 16) * 2
    AUXW = njtot + njtot + GMAX * njtot + GMAX + ES

    lT = nc.dram_tensor("lT", [C, RB], f32, kind="ExternalInput").ap()
    idx16 = nc.dram_tensor("idx16", [P, ncols_c + ncols_s], i16,
                           kind="ExternalInput").ap()
    aux = nc.dram_tensor("aux", [P, AUXW], f32, kind="ExternalInput").ap()
    out = nc.dram_tensor("out", [1, 1], f32, kind="ExternalOutput").ap()

    with tile.TileContext(nc) as tc:
        with tc.tile_pool(name="sb", bufs=1) as sb:
            # --- tiles ---
            dummy_idx = sb.tile([P, 1], i16)
            gdummy = sb.tile([P, ES], f32)
            idx16_t = sb.tile([P, ncols_c + ncols_s], i16)
            aux_t = sb.tile([P, AUXW], f32)
            gsub = [sb.tile([P, snj * ES], f32, name=f"gsub{si}")
                    for si, snj in enumerate(sub0 + sub1)]
            gsamp = sb.tile([P, 2 * ES], f32)
            ht = sb.tile([HP, HB * RB], f32)
            msk = sb.tile([P, njtot * ES], f32)
            val = sb.tile([P, njtot], f32)

            # --- early memsets (vector) + input DMAs (HWDGE rings) ---
            nc.vector.memset(dummy_idx[:, :], 0)
            nc.vector.memset(gsamp[:, :], -50.0)
            nc.sync.dma_start(out=idx16_t[:, :], in_=idx16[:, :])
            nc.sync.dma_start(out=aux_t[:, :], in_=aux[:, :])

            o = 0
            offt_t = aux_t[:, o:o + njtot]; o += njtot
            wcorr_t = aux_t[:, o:o + njtot]; o += njtot
            wg_t = aux_t[:, o:o + GMAX * njtot]; o += GMAX * njtot
            rcnt_t = aux_t[:, o:o + GMAX]; o += GMAX
            iota_t = aux_t[:, o:o + ES]; o += ES

            # head DMA split across both HWDGE rings
            hsrc = lT[:HEAD, :].rearrange("(p j) c -> p (j c)", j=HB)
            half = HB * RB // 2
            nc.scalar.dma_start(out=ht[:, :half], in_=hsrc[:, :half])
            nc.sync.dma_start(out=ht[:, half:], in_=hsrc[:, half:])

            # --- gathers on gpsimd (dummy first: pays the mlp IRAM load) ---
            nc.gpsimd.dma_gather(
                out_ap=gdummy[:, :].rearrange("p (j e) -> p j e", e=ES),
                in_ap=lT[0:16, :], idxs_ap=dummy_idx[:, :],
                num_idxs=16, num_idxs_reg=16, elem_size=ES,
                single_packet=False)

            oc = ncols_c
            for w, ns in ((0, ns0), (1, ns1)):
                if ns > 0:
                    lo = 0 if w == 0 else LO1
                    nc.gpsimd.dma_gather(
                        out_ap=gsamp[:, w * ES:(w + 1) * ES].rearrange(
                            "p (j e) -> p j e", e=ES),
                        in_ap=lT[lo:lo + WIN, :],
                        idxs_ap=idx16_t[:, oc:oc + 8],
                        num_idxs=128, num_idxs_reg=ns, elem_size=ES,
                        single_packet=False)
                oc += 8

            o16 = 0
            si = 0
            for w, subs in ((0, sub0), (1, sub1)):
                lo = 0 if w == 0 else LO1
                for snj in subs:
                    nn = snj * P // 16
                    nc.gpsimd.dma_gather(
                        out_ap=gsub[si][:, :].rearrange(
                            "p (j e) -> p j e", e=ES),
                        in_ap=lT[lo:lo + WIN, :],
                        idxs_ap=idx16_t[:, o16:o16 + nn],
                        num_idxs=snj * P, num_idxs_reg=snj * P, elem_size=ES,
                        single_packet=False)
                    o16 += nn
                    si += 1

            # --- scalar: sampled softplus first, then head ---
            sacc = sb.tile([P, 1], f32)
            nc.scalar.activation(gsamp[:, :], gsamp[:, :], AF.Exp)
            e_h = nc.scalar.activation(ht[:, :], ht[:, :], AF.Exp)
            nc.scalar.activation(gsamp[:, :], gsamp[:, :], AF.Ln, bias=1.0,
                                 accum_out=sacc[:, :])
            hacc = sb.tile([HP, 1], f32)
            ln_h = nc.scalar.activation(ht[:, :], ht[:, :], AF.Ln, bias=1.0,
                                        accum_out=hacc[:, :])

            # --- vector: eq masks early, then per-sub extract ---
            nc.vector.tensor_tensor(
                out=msk[:, :].rearrange("p (j e) -> p j e", e=ES),
                in0=iota_t.unsqueeze(1).to_broadcast([P, njtot, ES]),
                in1=offt_t.unsqueeze(2).to_broadcast([P, njtot, ES]),
                op=OP.is_equal)
            jo = 0
            for si, snj in enumerate(sub0 + sub1):
                nc.vector.tensor_tensor(
                    msk[:, jo * ES:(jo + snj) * ES],
                    msk[:, jo * ES:(jo + snj) * ES],
                    gsub[si][:, :], op=OP.mult)
                nc.vector.tensor_reduce(
                    val[:, jo:jo + snj],
                    msk[:, jo * ES:(jo + snj) * ES].rearrange(
                        "p (j e) -> p j e", e=ES),
                    AX.X, OP.add)
                jo += snj

            # dummy exp: reload the exp table during the gather window
            dummy = sb.tile([1, 1], f32)
            dex = nc.scalar.activation(dummy[:, :], hacc[0:1, :1], AF.Exp,
                                       scale=0.0)

            # --- term1: per-row avg over candidates ---
            scr2 = sb.tile([P, GMAX * njtot], f32)
            for g in range(GMAX):
                nc.vector.tensor_tensor(
                    scr2[:, g * njtot:(g + 1) * njtot],
                    wg_t[:, g * njtot:(g + 1) * njtot], val[:, :], op=OP.mult)
            csum = sb.tile([P, GMAX], f32)
            nc.vector.tensor_reduce(
                csum[:, :],
                scr2[:, :].rearrange("p (g j) -> p g j", g=GMAX),
                AX.X, OP.add)
            avg = sb.tile([P, GMAX], f32)
            nc.vector.tensor_tensor(avg[:, :], csum[:, :], rcnt_t, op=OP.mult)

            # --- late activations ---
            ce = sb.tile([P, njtot], f32)
            nc.scalar.activation(ce[:, :], val[:, :], AF.Exp)
            ae = sb.tile([P, GMAX], f32)
            nc.scalar.activation(ae[:, :], avg[:, :], AF.Exp, scale=-1.0)
            spl = sb.tile([P, njtot], f32)
            nc.scalar.activation(spl[:, :], ce[:, :], AF.Ln, bias=1.0)
            t1c = sb.tile([P, 1], f32)
            t1 = sb.tile([P, GMAX], f32)
            nc.scalar.activation(t1[:, :], ae[:, :], AF.Ln, bias=1.0,
                                 accum_out=t1c[:, :])

            # --- combine ---
            corr = sb.tile([P, 1], f32)
            scr3 = sb.tile([P, njtot], f32)
            nc.vector.tensor_tensor(scr3[:, :], wcorr_t, spl[:, :], op=OP.mult)
            nc.vector.tensor_reduce(corr[:, :], scr3[:, :], AX.X, OP.add)

            total = sb.tile([P, 1], f32)
            nc.vector.tensor_tensor(total[:, :], t1c[:, :], corr[:, :],
                                    op=OP.add)
            sacc2 = sb.tile([P, 1], f32)
            nc.vector.tensor_scalar_mul(sacc2[:, :], sacc[:, :], TSCALE)
            nc.vector.tensor_tensor(total[:, :], total[:, :], sacc2[:, :],
                                    op=OP.add)
            nc.vector.tensor_tensor(total[:HP, :], total[:HP, :], hacc[:, :],
                                    op=OP.add)

            gtot = sb.tile([P, 1], f32)
            nc.gpsimd.partition_all_reduce(gtot[:, :], total[:, :],
                                           channels=P,
                                           reduce_op=bass_isa.ReduceOp.add)
            res = sb.tile([1, 1], f32)
            nc.vector.tensor_scalar_mul(res[:, :], gtot[0:1, :], 1.0 / B)
            nc.sync.dma_start(out=out[:, :], in_=res[:, :])

    nc.compile()
    return nc


def get_graph(meta, enable_asserts=False):
    key = (meta, enable_asserts)
    if key not in _CACHE:
        _CACHE[key] = _build(meta, enable_asserts=enable_asserts)
    return _CACHE[key]


def run(logits, candidates, sampled_indices, trace=False, **kw):
    """Returns (scalar float32 loss, BassKernelResults)."""
    from concourse.bass_utils import run_bass_kernel_spmd

    in_maps, meta = prep_inputs(logits, candidates, sampled_indices)
    nc = get_graph(meta)
    res = run_bass_kernel_spmd(nc, in_maps, core_ids=list(range(NCORES)),
                               trace=trace, **kw)
    partials = [r["out"].reshape(()) for r in res.results]
    loss = np.float32(np.sum(np.stack(partials), dtype=np.float64))
    return loss, res


def kernel(logits, candidates, sampled_indices):
    loss, _ = run(logits, candidates, sampled_indices, trace=False)
    return loss


# revision 4
# speedup vs baseline: 1.1000x; 1.1000x over previous
"""AdaptiveCLPL loss on 8 TRN2 NeuronCores (Bass/Tile) — v3.

loss = mean_b [ psi(avg_cand) + sum_head psi(-l)*(1-mask) + ts*sum_samp psi(-l)*(1-is_cand) ]
with psi(u) = softplus(-u) = Ln(Exp(-u)+1) (no native softplus table).

Decomposition (only term1 is per-row nonlinear; everything else sums):
  total = sum_b softplus(-avg_b)
        + [sum_{head block} softplus(l)    - sum_k uniq*inhead*softplus(l_cand)]
        + ts*[sum_{sampled rows} softplus(l) - sum_k uniq*mult*softplus(l_cand)]

Per-core layout: transposed batch shard lT = logits[rows_perm].T ([C, RB]
row-major); every lT row is a 1KB chunk addressed by class. Candidate values
come from dma_gather (one 1KB descriptor per candidate). Key points:
  - overlapping int16 windows [0,32768) and [C-32768, C): candidates in the
    overlap go to either window, so every partition holds EXACTLY nj0+nj1
    candidate slots -> zero descriptor padding (2560 descriptors, the floor).
  - sampled rows ride the same two gather calls as extra trailing indices
    (num_idxs = 1280+ns_w), replacing the slow gpsimd indirect DMA. Each
    gather call has ~1.4us fixed cost, so only 2 real calls are used.
  - a 16-idx dummy gather issued first pays the gpsimd 'mlp' library IRAM
    load while the idx/aux DMAs are in flight; the 2MB head DMA is gated
    behind it so the library image isn't bandwidth-starved.
  - act tables are doctored at compile time so Exp and Ln resolve to the one
    table set that contains both -> a single ACT_TABLE_LOAD, no swaps.
  - rows are packed 2 per partition; the shard column of row (p,g) is 2p+g.
"""

import numpy as np

B, C, K = 2048, 50000, 10
HEAD, S = 2000, 100
TSCALE = float(C - HEAD) / float(S)  # 480.0
NCORES = 8
RB = B // NCORES  # 256 rows per core
P = 128
HP = 125          # head tile partitions; 2000 rows = 125 * 16
HB = HEAD // HP   # 16 blocks -> 16KB contiguous per partition
ES = 256          # chunk = one lT row (1KB)
WIN = 32768
LO1 = C - WIN     # 17232; window1 = [LO1, C)
GMAX = 2          # exactly 2 rows per partition

_CACHE = {}


def _pack_rows(h0, h1, nj_target, rng):
    """Pair 2*P rows into P partitions s.t. per-partition hard-window counts
    stay <= nj_target. Returns part[r] in [0,P)."""
    nrows = len(h0)
    order = np.argsort(-h0, kind="stable")
    part = np.zeros(nrows, np.int64)
    for i in range(P):
        part[order[i]] = i
        part[order[nrows - 1 - i]] = i
    H0 = np.bincount(part, weights=h0, minlength=P)
    H1 = np.bincount(part, weights=h1, minlength=P)

    def viol(a0, a1):
        return max(a0 - nj_target, 0) + max(a1 - nj_target, 0)

    cur = sum(viol(H0[p], H1[p]) for p in range(P))
    it = 0
    while cur > 0 and it < 20000:
        it += 1
        a, b = rng.integers(0, nrows, 2)
        pa, pb = part[a], part[b]
        if pa == pb:
            continue
        old = viol(H0[pa], H1[pa]) + viol(H0[pb], H1[pb])
        H0[pa] += h0[b] - h0[a]; H1[pa] += h1[b] - h1[a]
        H0[pb] += h0[a] - h0[b]; H1[pb] += h1[a] - h1[b]
        new = viol(H0[pa], H1[pa]) + viol(H0[pb], H1[pb])
        if new <= old:
            part[a], part[b] = pb, pa
            cur += new - old
        else:
            H0[pa] -= h0[b] - h0[a]; H1[pa] -= h1[b] - h1[a]
            H0[pb] -= h0[a] - h0[b]; H1[pb] -= h1[a] - h1[b]
    return part, cur == 0


def prep_inputs(logits, candidates, sampled_indices):
    """Full inputs -> (in_maps, meta). Host work is sharding + index math only."""
    logits = np.asarray(logits)
    candidates = np.asarray(candidates)
    sampled_indices = np.asarray(sampled_indices)
    assert logits.shape == (B, C) and candidates.shape == (B, K)
    srow = (HEAD + sampled_indices.astype(np.int64)).astype(np.int64)  # [S]
    svals, scounts = np.unique(srow, return_counts=True)
    smult = dict(zip(svals.tolist(), scounts.tolist()))

    # sampled rows -> windows (balance the flexible ones)
    s_w = np.where(srow < LO1, 0, np.where(srow >= WIN, 1, -1))
    flex = np.where(s_w < 0)[0]
    n0 = int((s_w == 0).sum())
    n1 = int((s_w == 1).sum())
    for j in flex:
        if n0 <= n1:
            s_w[j] = 0; n0 += 1
        else:
            s_w[j] = 1; n1 += 1
    ns0, ns1 = n0, n1
    sidx_w = [srow[s_w == 0] - 0, srow[s_w == 1] - LO1]

    rng = np.random.default_rng(12345)
    cores = []
    nj_need = [1, 1]
    for i in range(NCORES):
        rows = slice(i * RB, (i + 1) * RB)
        cand = candidates[rows].astype(np.int64)          # [RB, K]
        valid = cand >= 0
        uniq = valid.copy()
        for k in range(1, K):
            dup = (cand[:, :k] == cand[:, k:k + 1]).any(axis=1)
            uniq[:, k] &= ~dup
        uniqf = uniq.astype(np.float32)
        cnt = np.maximum((uniq & valid).sum(axis=1), 1).astype(np.float32)
        inhead = (cand < HEAD).astype(np.float32)
        mult = np.vectorize(lambda c: smult.get(int(c), 0))(cand).astype(np.float32)
        wcorr_rk = -uniqf * (inhead + TSCALE * mult)      # [RB, K]

        h0 = (valid & (cand < LO1)).sum(axis=1)
        h1 = (valid & (cand >= WIN)).sum(axis=1)
        part, ok = _pack_rows(h0.astype(np.int64), h1.astype(np.int64), K, rng)
        grp = np.zeros(RB, np.int64)
        seen = {}
        for r in range(RB):
            p = int(part[r])
            grp[r] = seen.get(p, 0)
            seen[p] = grp[r] + 1
        assert max(seen.values()) <= GMAX

        # window assignment per candidate
        cw = np.full((RB, K), -1, np.int64)
        cw[valid & (cand < LO1)] = 0
        cw[valid & (cand >= WIN)] = 1
        H0 = np.bincount(part, weights=(cw == 0).sum(1), minlength=P).astype(np.int64)
        for r in range(RB):
            p = int(part[r])
            for k in range(K):
                if valid[r, k] and cw[r, k] < 0:
                    if H0[p] < K:
                        cw[r, k] = 0; H0[p] += 1
                    else:
                        cw[r, k] = 1
        W0c = np.bincount(part, weights=(cw == 0).sum(1), minlength=P).astype(np.int64)
        W1c = np.bincount(part, weights=(cw == 1).sum(1), minlength=P).astype(np.int64)
        nj_need[0] = max(nj_need[0], int(W0c.max()))
        nj_need[1] = max(nj_need[1], int(W1c.max()))
        cores.append((cand, valid, uniqf, cnt, wcorr_rk, part, grp, cw))

    nj0, nj1 = nj_need
    njtot = nj0 + nj1
    meta = (nj0, nj1, ns0, ns1)

    # idx16 layout: [w0 cand+samp | w1 cand+samp] wrapped per call
    ni = [nj0 * P + ns0, nj1 * P + ns1]          # num_idxs per call
    ncols = [(-(-n // 16)) for n in ni]
    AUXW = njtot + njtot + GMAX * njtot + GMAX + ES

    in_maps = []
    for i in range(NCORES):
        cand, valid, uniqf, cnt, wcorr_rk, part, grp, cw = cores[i]
        rows = slice(i * RB, (i + 1) * RB)
        col = (2 * part + grp).astype(np.int64)
        perm = np.zeros(RB, np.int64)
        perm[col] = np.arange(RB)
        lT = np.ascontiguousarray(logits[rows][perm].T.astype(np.float32, copy=False))

        idxs = [np.zeros(nj0 * P, np.int16), np.zeros(nj1 * P, np.int16)]
        offt = np.full((P, njtot), -1.0, np.float32)
        wcorr = np.zeros((P, njtot), np.float32)
        wg = np.zeros((P, GMAX * njtot), np.float32)
        fill = np.zeros((P, 2), np.int64)
        base_j = [0, nj0]
        base_lo = [0, LO1]
        for r in range(RB):
            p, g = int(part[r]), int(grp[r])
            for k in range(K):
                if not valid[r, k]:
                    continue
                w = int(cw[r, k])
                j = int(fill[p, w]); fill[p, w] += 1
                idxs[w][j * P + p] = cand[r, k] - base_lo[w]
                jj = base_j[w] + j
                offt[p, jj] = float(col[r])
                wcorr[p, jj] = wcorr_rk[r, k]
                wg[p, g * njtot + jj] = uniqf[r, k]
        rcnt = np.zeros((P, GMAX), np.float32)
        rcnt[part, grp] = 1.0 / cnt
        iota = np.broadcast_to(np.arange(ES, dtype=np.float32), (P, ES)).copy()

        cols_out = []
        for w in (0, 1):
            flat = np.zeros(ncols[w] * 16, np.int16)
            flat[:len(idxs[w])] = idxs[w]
            flat[len(idxs[w]):len(idxs[w]) + len(sidx_w[w])] = (
                sidx_w[w].astype(np.int16))
            wrapped = flat.reshape(ncols[w], 16).T
            cols_out.append(np.tile(wrapped, (8, 1)))
        idx16 = np.ascontiguousarray(np.concatenate(cols_out, axis=1))
        assert idx16.shape == (P, sum(ncols))

        auxcat = np.ascontiguousarray(np.concatenate(
            [offt, wcorr, wg, rcnt, iota], axis=1))
        assert auxcat.shape == (P, AUXW)
        in_maps.append({"lT": lT, "idx16": idx16, "aux": auxcat})
    return in_maps, meta


def _act_table_patch():
    """Context manager: make Exp and Ln resolve only to the one act-func set
    ('natural_log_exp_and_others') that holds both, so the compile-time table
    placement emits a single ACT_TABLE_LOAD instead of swapping per phase."""
    import contextlib
    from concourse import hw_specs, mybir

    @contextlib.contextmanager
    def ctx():
        real = hw_specs.get_activation_tables
        AF = mybir.ActivationFunctionType

        def doctored(arch):
            tabs = {k: set(v) for k, v in real(arch).items()}
            if any(AF.Exp in v and AF.Ln in v for v in tabs.values()):
                for name, s in tabs.items():
                    if not (AF.Exp in s and AF.Ln in s):
                        s.discard(AF.Exp)
                        s.discard(AF.Ln)
            return tabs

        hw_specs.get_activation_tables = doctored
        try:
            yield
        finally:
            hw_specs.get_activation_tables = real

    return ctx()


def _build(meta, enable_asserts=False):
    import concourse.bass as bass
    import concourse.tile as tile
    from concourse import bacc, bass_isa, mybir
    from concourse.bass import _add_dep_helper

    nj0, nj1, ns0, ns1 = meta
    njtot = nj0 + nj1
    njs = [nj0, nj1]
    nss = [ns0, ns1]
    ni = [nj0 * P + ns0, nj1 * P + ns1]
    ncols = [(-(-n // 16)) for n in ni]
    AUXW = njtot + njtot + GMAX * njtot + GMAX + ES

    f32 = mybir.dt.float32
    i16 = mybir.dt.int16
    AF = mybir.ActivationFunctionType
    OP = mybir.AluOpType
    AX = mybir.AxisListType

    nc = bacc.Bacc(
        "TRN2",
        target_bir_lowering=False,
        debug=False,
        enable_asserts=enable_asserts,
        num_devices=NCORES,
    )

    lT = nc.dram_tensor("lT", [C, RB], f32, kind="ExternalInput").ap()
    idx16 = nc.dram_tensor("idx16", [P, sum(ncols)], i16,
                           kind="ExternalInput").ap()
    aux = nc.dram_tensor("aux", [P, AUXW], f32, kind="ExternalInput").ap()
    out = nc.dram_tensor("out", [1, 1], f32, kind="ExternalOutput").ap()

    with tile.TileContext(nc) as tc:
        with tc.tile_pool(name="sb", bufs=1) as sb:
            # --- tiles ---
            dummy_idx = sb.tile([P, 1], i16)
            gdummy = sb.tile([P, ES], f32)
            idx16_t = sb.tile([P, sum(ncols)], i16)
            aux_t = sb.tile([P, AUXW], f32)
            gwin = [sb.tile([P, (njs[w] + 1) * ES], f32, name=f"gwin{w}")
                    for w in (0, 1)]
            ht = sb.tile([HP, HB * RB], f32)
            msk = sb.tile([P, njtot * ES], f32)
            val = sb.tile([P, njtot], f32)

            # --- early memsets (vector) + small input DMAs (scalar ring) ---
            nc.vector.memset(dummy_idx[:, :], 0)
            nc.vector.memset(gwin[0][:, njs[0] * ES:], -50.0)
            nc.vector.memset(gwin[1][:, njs[1] * ES:], -50.0)
            nc.scalar.dma_start(out=idx16_t[:, :], in_=idx16[:, :])
            nc.scalar.dma_start(out=aux_t[:, :], in_=aux[:, :])

            o = 0
            offt_t = aux_t[:, o:o + njtot]; o += njtot
            wcorr_t = aux_t[:, o:o + njtot]; o += njtot
            wg_t = aux_t[:, o:o + GMAX * njtot]; o += GMAX * njtot
            rcnt_t = aux_t[:, o:o + GMAX]; o += GMAX
            iota_t = aux_t[:, o:o + ES]; o += ES

            # --- gathers on gpsimd (dummy first: pays the mlp IRAM load) ---
            gdum = nc.gpsimd.dma_gather(
                out_ap=gdummy[:, :].rearrange("p (j e) -> p j e", e=ES),
                in_ap=lT[0:16, :], idxs_ap=dummy_idx[:, :],
                num_idxs=16, num_idxs_reg=16, elem_size=ES,
                single_packet=False)

            oc = 0
            gcalls = []
            for w in (0, 1):
                lo = 0 if w == 0 else LO1
                g = nc.gpsimd.dma_gather(
                    out_ap=gwin[w][:, :].rearrange("p (j e) -> p j e", e=ES),
                    in_ap=lT[lo:lo + WIN, :],
                    idxs_ap=idx16_t[:, oc:oc + ncols[w]],
                    num_idxs=ni[w], num_idxs_reg=ni[w], elem_size=ES,
                    single_packet=False)
                gcalls.append(g)
                oc += ncols[w]

            # --- head DMA on sync ring, gated behind the library load so the
            # ucode image isn't bandwidth-starved ---
            hsrc = lT[:HEAD, :].rearrange("(p j) c -> p (j c)", j=HB)
            half = HB * RB // 2
            d_h0 = nc.sync.dma_start(out=ht[:, :half], in_=hsrc[:, :half])
            d_h1 = nc.sync.dma_start(out=ht[:, half:], in_=hsrc[:, half:])
            for d in (d_h0, d_h1):
                _add_dep_helper(d.ins, gdum.ins, sync=True,
                                reason="head DMA after gpsimd lib load")

            # --- vector: eq masks early, then per-window extract ---
            nc.vector.tensor_tensor(
                out=msk[:, :].rearrange("p (j e) -> p j e", e=ES),
                in0=iota_t.unsqueeze(1).to_broadcast([P, njtot, ES]),
                in1=offt_t.unsqueeze(2).to_broadcast([P, njtot, ES]),
                op=OP.is_equal)
            jo = 0
            for w in (0, 1):
                nc.vector.tensor_tensor(
                    msk[:, jo * ES:(jo + njs[w]) * ES],
                    msk[:, jo * ES:(jo + njs[w]) * ES],
                    gwin[w][:, :njs[w] * ES], op=OP.mult)
                nc.vector.tensor_reduce(
                    val[:, jo:jo + njs[w]],
                    msk[:, jo * ES:(jo + njs[w]) * ES].rearrange(
                        "p (j e) -> p j e", e=ES),
                    AX.X, OP.add)
                jo += njs[w]

            # --- scalar: softplus everywhere (single Exp+Ln table set) ---
            sacc = [sb.tile([P, 1], f32, name=f"sacc{w}") for w in (0, 1)]
            e_s = [nc.scalar.activation(gwin[w][:, njs[w] * ES:],
                                        gwin[w][:, njs[w] * ES:], AF.Exp)
                   for w in (0, 1)]
            l_s = [nc.scalar.activation(gwin[w][:, njs[w] * ES:],
                                        gwin[w][:, njs[w] * ES:], AF.Ln,
                                        bias=1.0, accum_out=sacc[w][:, :])
                   for w in (0, 1)]
            hacc = sb.tile([HP, 1], f32)
            nc.scalar.activation(ht[:, :], ht[:, :], AF.Exp)
            nc.scalar.activation(ht[:, :], ht[:, :], AF.Ln, bias=1.0,
                                 accum_out=hacc[:, :])

            # --- term1: per-row avg over candidates ---
            scr2 = sb.tile([P, GMAX * njtot], f32)
            for g in range(GMAX):
                nc.vector.tensor_tensor(
                    scr2[:, g * njtot:(g + 1) * njtot],
                    wg_t[:, g * njtot:(g + 1) * njtot], val[:, :], op=OP.mult)
            csum = sb.tile([P, GMAX], f32)
            nc.vector.tensor_reduce(
                csum[:, :],
                scr2[:, :].rearrange("p (g j) -> p g j", g=GMAX),
                AX.X, OP.add)
            avg = sb.tile([P, GMAX], f32)
            nc.vector.tensor_tensor(avg[:, :], csum[:, :], rcnt_t, op=OP.mult)

            # --- late activations ---
            ce = sb.tile([P, njtot], f32)
            nc.scalar.activation(ce[:, :], val[:, :], AF.Exp)
            ae = sb.tile([P, GMAX], f32)
            nc.scalar.activation(ae[:, :], avg[:, :], AF.Exp, scale=-1.0)
            spl = sb.tile([P, njtot], f32)
            nc.scalar.activation(spl[:, :], ce[:, :], AF.Ln, bias=1.0)
            t1c = sb.tile([P, 1], f32)
            t1 = sb.tile([P, GMAX], f32)
            nc.scalar.activation(t1[:, :], ae[:, :], AF.Ln, bias=1.0,
                                 accum_out=t1c[:, :])

            # --- combine ---
            corr = sb.tile([P, 1], f32)
            scr3 = sb.tile([P, njtot], f32)
            nc.vector.tensor_tensor(scr3[:, :], wcorr_t, spl[:, :], op=OP.mult)
            nc.vector.tensor_reduce(corr[:, :], scr3[:, :], AX.X, OP.add)

            total = sb.tile([P, 1], f32)
            nc.vector.tensor_tensor(total[:, :], t1c[:, :], corr[:, :],
                                    op=OP.add)
            stot = sb.tile([P, 1], f32)
            nc.vector.tensor_tensor(stot[:, :], sacc[0][:, :], sacc[1][:, :],
                                    op=OP.add)
            sacc2 = sb.tile([P, 1], f32)
            nc.vector.tensor_scalar_mul(sacc2[:, :], stot[:, :], TSCALE)
            nc.vector.tensor_tensor(total[:, :], total[:, :], sacc2[:, :],
                                    op=OP.add)
            nc.vector.tensor_tensor(total[:HP, :], total[:HP, :], hacc[:, :],
                                    op=OP.add)

            gtot = sb.tile([P, 1], f32)
            nc.gpsimd.partition_all_reduce(gtot[:, :], total[:, :],
                                           channels=P,
                                           reduce_op=bass_isa.ReduceOp.add)
            res = sb.tile([1, 1], f32)
            nc.vector.tensor_scalar_mul(res[:, :], gtot[0:1, :], 1.0 / B)
            nc.sync.dma_start(out=out[:, :], in_=res[:, :])

    with _act_table_patch():
        nc.compile()
    return nc


def get_graph(meta, enable_asserts=False):
    key = (meta, enable_asserts)
    if key not in _CACHE:
        _CACHE[key] = _build(meta, enable_asserts=enable_asserts)
    return _CACHE[key]


def run(logits, candidates, sampled_indices, trace=False, **kw):
    """Returns (scalar float32 loss, BassKernelResults)."""
    from concourse.bass_utils import run_bass_kernel_spmd

    in_maps, meta = prep_inputs(logits, candidates, sampled_indices)
    nc = get_graph(meta)
    res = run_bass_kernel_spmd(nc, in_maps, core_ids=list(range(NCORES)),
                               trace=trace, **kw)
    partials = [r["out"].reshape(()) for r in res.results]
    loss = np.float32(np.sum(np.stack(partials), dtype=np.float64))
    return loss, res


def kernel(logits, candidates, sampled_indices):
    loss, _ = run(logits, candidates, sampled_indices, trace=False)
    return loss


# revision 7
# speedup vs baseline: 1.2018x; 1.0926x over previous
"""AdaptiveCLPL loss on 8 TRN2 NeuronCores (Bass/Tile) — v3.

loss = mean_b [ psi(avg_cand) + sum_head psi(-l)*(1-mask) + ts*sum_samp psi(-l)*(1-is_cand) ]
with psi(u) = softplus(-u) = Ln(Exp(-u)+1) (no native softplus table).

Decomposition (only term1 is per-row nonlinear; everything else sums):
  total = sum_b softplus(-avg_b)
        + [sum_{head block} softplus(l)    - sum_k uniq*inhead*softplus(l_cand)]
        + ts*[sum_{sampled rows} softplus(l) - sum_k uniq*mult*softplus(l_cand)]

Per-core layout: transposed batch shard lT = logits[rows_perm].T ([C, RB]
row-major); every lT row is a 1KB chunk addressed by class. Candidate values
come from dma_gather (one 1KB descriptor per candidate). Key points:
  - overlapping int16 windows [0,32768) and [C-32768, C): candidates in the
    overlap go to either window, so every partition holds EXACTLY nj0+nj1
    candidate slots -> zero descriptor padding (2560 descriptors, the floor).
  - sampled rows ride the same two gather calls as extra trailing indices
    (num_idxs = 1280+ns_w), replacing the slow gpsimd indirect DMA. Each
    gather call has ~1.4us fixed cost, so only 2 real calls are used.
  - a 16-idx dummy gather issued first pays the gpsimd 'mlp' library IRAM
    load while the idx/aux DMAs are in flight; the 2MB head DMA is gated
    behind it so the library image isn't bandwidth-starved.
  - act tables are doctored at compile time so Exp and Ln resolve to the one
    table set that contains both -> a single ACT_TABLE_LOAD, no swaps.
  - rows are packed 2 per partition; the shard column of row (p,g) is 2p+g.
"""

import numpy as np

B, C, K = 2048, 50000, 10
HEAD, S = 2000, 100
TSCALE = float(C - HEAD) / float(S)  # 480.0
NCORES = 8
RB = B // NCORES  # 256 rows per core
P = 128
HP = 125          # head tile partitions; 2000 rows = 125 * 16
HB = HEAD // HP   # 16 blocks -> 16KB contiguous per partition
ES = 256          # chunk = one lT row (1KB)
WIN = 32768
LO1 = C - WIN     # 17232; window1 = [LO1, C)
GMAX = 2          # exactly 2 rows per partition

_CACHE = {}


def _pack_rows(h0, h1, nj_target, rng):
    """Pair 2*P rows into P partitions s.t. per-partition hard-window counts
    stay <= nj_target. Returns part[r] in [0,P)."""
    nrows = len(h0)
    order = np.argsort(-h0, kind="stable")
    part = np.zeros(nrows, np.int64)
    for i in range(P):
        part[order[i]] = i
        part[order[nrows - 1 - i]] = i
    H0 = np.bincount(part, weights=h0, minlength=P)
    H1 = np.bincount(part, weights=h1, minlength=P)

    def viol(a0, a1):
        return max(a0 - nj_target, 0) + max(a1 - nj_target, 0)

    cur = sum(viol(H0[p], H1[p]) for p in range(P))
    it = 0
    while cur > 0 and it < 20000:
        it += 1
        a, b = rng.integers(0, nrows, 2)
        pa, pb = part[a], part[b]
        if pa == pb:
            continue
        old = viol(H0[pa], H1[pa]) + viol(H0[pb], H1[pb])
        H0[pa] += h0[b] - h0[a]; H1[pa] += h1[b] - h1[a]
        H0[pb] += h0[a] - h0[b]; H1[pb] += h1[a] - h1[b]
        new = viol(H0[pa], H1[pa]) + viol(H0[pb], H1[pb])
        if new <= old:
            part[a], part[b] = pb, pa
            cur += new - old
        else:
            H0[pa] -= h0[b] - h0[a]; H1[pa] -= h1[b] - h1[a]
            H0[pb] -= h0[a] - h0[b]; H1[pb] -= h1[a] - h1[b]
    return part, cur == 0


def prep_inputs(logits, candidates, sampled_indices):
    """Full inputs -> (in_maps, meta). Host work is sharding + index math only."""
    logits = np.asarray(logits)
    candidates = np.asarray(candidates)
    sampled_indices = np.asarray(sampled_indices)
    assert logits.shape == (B, C) and candidates.shape == (B, K)
    srow = (HEAD + sampled_indices.astype(np.int64)).astype(np.int64)  # [S]
    svals, scounts = np.unique(srow, return_counts=True)
    smult = dict(zip(svals.tolist(), scounts.tolist()))

    # sampled rows -> windows (balance the flexible ones)
    s_w = np.where(srow < LO1, 0, np.where(srow >= WIN, 1, -1))
    flex = np.where(s_w < 0)[0]
    n0 = int((s_w == 0).sum())
    n1 = int((s_w == 1).sum())
    for j in flex:
        if n0 <= n1:
            s_w[j] = 0; n0 += 1
        else:
            s_w[j] = 1; n1 += 1
    ns0, ns1 = n0, n1
    sidx_w = [srow[s_w == 0] - 0, srow[s_w == 1] - LO1]

    rng = np.random.default_rng(12345)
    cores = []
    nj_need = [1, 1]
    for i in range(NCORES):
        rows = slice(i * RB, (i + 1) * RB)
        cand = candidates[rows].astype(np.int64)          # [RB, K]
        valid = cand >= 0
        uniq = valid.copy()
        for k in range(1, K):
            dup = (cand[:, :k] == cand[:, k:k + 1]).any(axis=1)
            uniq[:, k] &= ~dup
        uniqf = uniq.astype(np.float32)
        cnt = np.maximum((uniq & valid).sum(axis=1), 1).astype(np.float32)
        inhead = (cand < HEAD).astype(np.float32)
        mult = np.vectorize(lambda c: smult.get(int(c), 0))(cand).astype(np.float32)
        wcorr_rk = -uniqf * (inhead + TSCALE * mult)      # [RB, K]

        h0 = (valid & (cand < LO1)).sum(axis=1)
        h1 = (valid & (cand >= WIN)).sum(axis=1)
        part, ok = _pack_rows(h0.astype(np.int64), h1.astype(np.int64), K, rng)
        grp = np.zeros(RB, np.int64)
        seen = {}
        for r in range(RB):
            p = int(part[r])
            grp[r] = seen.get(p, 0)
            seen[p] = grp[r] + 1
        assert max(seen.values()) <= GMAX

        # window assignment per candidate
        cw = np.full((RB, K), -1, np.int64)
        cw[valid & (cand < LO1)] = 0
        cw[valid & (cand >= WIN)] = 1
        H0 = np.bincount(part, weights=(cw == 0).sum(1), minlength=P).astype(np.int64)
        for r in range(RB):
            p = int(part[r])
            for k in range(K):
                if valid[r, k] and cw[r, k] < 0:
                    if H0[p] < K:
                        cw[r, k] = 0; H0[p] += 1
                    else:
                        cw[r, k] = 1
        W0c = np.bincount(part, weights=(cw == 0).sum(1), minlength=P).astype(np.int64)
        W1c = np.bincount(part, weights=(cw == 1).sum(1), minlength=P).astype(np.int64)
        nj_need[0] = max(nj_need[0], int(W0c.max()))
        nj_need[1] = max(nj_need[1], int(W1c.max()))
        cores.append((cand, valid, uniqf, cnt, wcorr_rk, part, grp, cw))

    nj0, nj1 = nj_need
    njtot = nj0 + nj1
    meta = (nj0, nj1, ns0, ns1)

    # idx16 layout: [w0 cand+samp | w1 cand+samp] wrapped per call
    ni = [nj0 * P + ns0, nj1 * P + ns1]          # num_idxs per call
    ncols = [(-(-n // 16)) for n in ni]
    AUXW = njtot + njtot + GMAX * njtot + GMAX + ES

    in_maps = []
    for i in range(NCORES):
        cand, valid, uniqf, cnt, wcorr_rk, part, grp, cw = cores[i]
        rows = slice(i * RB, (i + 1) * RB)
        col = (2 * part + grp).astype(np.int64)
        perm = np.zeros(RB, np.int64)
        perm[col] = np.arange(RB)
        lT = np.ascontiguousarray(logits[rows][perm].T.astype(np.float32, copy=False))

        idxs = [np.zeros(nj0 * P, np.int16), np.zeros(nj1 * P, np.int16)]
        offt = np.full((P, njtot), -1.0, np.float32)
        wcorr = np.zeros((P, njtot), np.float32)
        wg = np.zeros((P, GMAX * njtot), np.float32)
        fill = np.zeros((P, 2), np.int64)
        base_j = [0, nj0]
        base_lo = [0, LO1]
        for r in range(RB):
            p, g = int(part[r]), int(grp[r])
            for k in range(K):
                if not valid[r, k]:
                    continue
                w = int(cw[r, k])
                j = int(fill[p, w]); fill[p, w] += 1
                idxs[w][j * P + p] = cand[r, k] - base_lo[w]
                jj = base_j[w] + j
                offt[p, jj] = float(col[r])
                wcorr[p, jj] = wcorr_rk[r, k]
                wg[p, g * njtot + jj] = uniqf[r, k]
        rcnt = np.zeros((P, GMAX), np.float32)
        rcnt[part, grp] = 1.0 / cnt
        iota = np.broadcast_to(np.arange(ES, dtype=np.float32), (P, ES)).copy()

        cols_out = []
        for w in (0, 1):
            flat = np.zeros(ncols[w] * 16, np.int16)
            flat[:len(idxs[w])] = idxs[w]
            flat[len(idxs[w]):len(idxs[w]) + len(sidx_w[w])] = (
                sidx_w[w].astype(np.int16))
            wrapped = flat.reshape(ncols[w], 16).T
            cols_out.append(np.tile(wrapped, (8, 1)))
        idx16 = np.ascontiguousarray(np.concatenate(cols_out, axis=1))
        assert idx16.shape == (P, sum(ncols))

        auxcat = np.ascontiguousarray(np.concatenate(
            [offt, wcorr, wg, rcnt, iota], axis=1))
        assert auxcat.shape == (P, AUXW)
        in_maps.append({"lT": lT, "idx16": idx16, "aux": auxcat})
    return in_maps, meta


def _act_table_patch():
    """Context manager: make Exp and Ln resolve only to the one act-func set
    ('natural_log_exp_and_others') that holds both, so the compile-time table
    placement emits a single ACT_TABLE_LOAD instead of swapping per phase."""
    import contextlib
    from concourse import hw_specs, mybir

    import contextlib as _ctl
    from concourse import bacc as _bacc

    @_ctl.contextmanager
    def ctx():
        real = hw_specs.get_activation_tables
        AF = mybir.ActivationFunctionType

        def doctored(arch):
            tabs = {k: set(v) for k, v in real(arch).items()}
            if any(AF.Exp in v and AF.Ln in v for v in tabs.values()):
                for name, s in tabs.items():
                    if not (AF.Exp in s and AF.Ln in s):
                        s.discard(AF.Exp)
                        s.discard(AF.Ln)
            return tabs

        hw_specs.get_activation_tables = doctored
        _bacc.get_activation_tables = doctored
        try:
            yield
        finally:
            hw_specs.get_activation_tables = real
            _bacc.get_activation_tables = real

    return ctx()


def _build(meta, enable_asserts=False):
    import concourse.bass as bass
    import concourse.tile as tile
    from concourse import bacc, bass_isa, mybir
    from concourse.bass import _add_dep_helper

    nj0, nj1, ns0, ns1 = meta
    njtot = nj0 + nj1
    njs = [nj0, nj1]
    nss = [ns0, ns1]
    ni = [nj0 * P + ns0, nj1 * P + ns1]
    ncols = [(-(-n // 16)) for n in ni]
    AUXW = njtot + njtot + GMAX * njtot + GMAX + ES

    f32 = mybir.dt.float32
    i16 = mybir.dt.int16
    AF = mybir.ActivationFunctionType
    OP = mybir.AluOpType
    AX = mybir.AxisListType

    nc = bacc.Bacc(
        "TRN2",
        target_bir_lowering=False,
        debug=False,
        enable_asserts=enable_asserts,
        num_devices=NCORES,
        num_swdge_queues=2,
    )

    lT = nc.dram_tensor("lT", [C, RB], f32, kind="ExternalInput").ap()
    idx16 = nc.dram_tensor("idx16", [P, sum(ncols)], i16,
                           kind="ExternalInput").ap()
    aux = nc.dram_tensor("aux", [P, AUXW], f32, kind="ExternalInput").ap()
    out = nc.dram_tensor("out", [1, 1], f32, kind="ExternalOutput").ap()

    with tile.TileContext(nc) as tc:
        with tc.tile_pool(name="sb", bufs=1) as sb:
            # --- tiles ---
            dummy_idx = sb.tile([P, 1], i16)
            gdummy = sb.tile([P, ES], f32)
            idx16_t = sb.tile([P, sum(ncols)], i16)
            aux_t = sb.tile([P, AUXW], f32)
            gwin = [sb.tile([P, (njs[w] + 1) * ES], f32, name=f"gwin{w}")
                    for w in (0, 1)]
            ht = sb.tile([HP, HB * RB], f32)
            msk = sb.tile([P, njtot * ES], f32)
            val = sb.tile([P, njtot], f32)

            # --- early memsets (vector) + small input DMAs (scalar ring) ---
            nc.vector.memset(dummy_idx[:, :], 0)
            nc.vector.memset(gwin[0][:, njs[0] * ES:], -50.0)
            nc.vector.memset(gwin[1][:, njs[1] * ES:], -50.0)
            nc.scalar.dma_start(out=idx16_t[:, :], in_=idx16[:, :])
            nc.scalar.dma_start(out=aux_t[:, :], in_=aux[:, :])

            o = 0
            offt_t = aux_t[:, o:o + njtot]; o += njtot
            wcorr_t = aux_t[:, o:o + njtot]; o += njtot
            wg_t = aux_t[:, o:o + GMAX * njtot]; o += GMAX * njtot
            rcnt_t = aux_t[:, o:o + GMAX]; o += GMAX
            iota_t = aux_t[:, o:o + ES]; o += ES

            # --- gathers on gpsimd (dummy first: pays the mlp IRAM load) ---
            gdum = nc.gpsimd.dma_gather(
                out_ap=gdummy[:, :].rearrange("p (j e) -> p j e", e=ES),
                in_ap=lT[0:16, :], idxs_ap=dummy_idx[:, :],
                num_idxs=16, num_idxs_reg=16, elem_size=ES,
                single_packet=False)

            oc = 0
            gcalls = []
            for w in (0, 1):
                lo = 0 if w == 0 else LO1
                g = nc.gpsimd.dma_gather(
                    out_ap=gwin[w][:, :].rearrange("p (j e) -> p j e", e=ES),
                    in_ap=lT[lo:lo + WIN, :],
                    idxs_ap=idx16_t[:, oc:oc + ncols[w]],
                    num_idxs=ni[w], num_idxs_reg=ni[w], elem_size=ES,
                    single_packet=False, queue_num=w)
                gcalls.append(g)
                oc += ncols[w]

            # --- head DMA on sync ring, gated behind the library load so the
            # ucode image isn't bandwidth-starved ---
            hsrc = lT[:HEAD, :].rearrange("(p j) c -> p (j c)", j=HB)
            half = HB * RB // 2
            d_h0 = nc.sync.dma_start(out=ht[:, :half], in_=hsrc[:, :half])
            d_h1 = nc.sync.dma_start(out=ht[:, half:], in_=hsrc[:, half:])
            for d in (d_h0, d_h1):
                _add_dep_helper(d.ins, gdum.ins, sync=True,
                                reason="head DMA after gpsimd lib load")

            # --- vector: eq masks early, then per-window extract ---
            nc.vector.tensor_tensor(
                out=msk[:, :].rearrange("p (j e) -> p j e", e=ES),
                in0=iota_t.unsqueeze(1).to_broadcast([P, njtot, ES]),
                in1=offt_t.unsqueeze(2).to_broadcast([P, njtot, ES]),
                op=OP.is_equal)
            jo = 0
            for w in (0, 1):
                nc.vector.tensor_tensor(
                    msk[:, jo * ES:(jo + njs[w]) * ES],
                    msk[:, jo * ES:(jo + njs[w]) * ES],
                    gwin[w][:, :njs[w] * ES], op=OP.mult)
                nc.vector.tensor_reduce(
                    val[:, jo:jo + njs[w]],
                    msk[:, jo * ES:(jo + njs[w]) * ES].rearrange(
                        "p (j e) -> p j e", e=ES),
                    AX.X, OP.add)
                jo += njs[w]

            # --- scalar: softplus everywhere (single Exp+Ln table set) ---
            sacc = [sb.tile([P, 1], f32, name=f"sacc{w}") for w in (0, 1)]
            e_s = [nc.scalar.activation(gwin[w][:, njs[w] * ES:],
                                        gwin[w][:, njs[w] * ES:], AF.Exp)
                   for w in (0, 1)]
            l_s = [nc.scalar.activation(gwin[w][:, njs[w] * ES:],
                                        gwin[w][:, njs[w] * ES:], AF.Ln,
                                        bias=1.0, accum_out=sacc[w][:, :])
                   for w in (0, 1)]
            hacc = sb.tile([HP, 1], f32)
            nc.scalar.activation(ht[:, :], ht[:, :], AF.Exp)
            nc.scalar.activation(ht[:, :], ht[:, :], AF.Ln, bias=1.0,
                                 accum_out=hacc[:, :])

            # --- term1: per-row avg over candidates ---
            scr2 = sb.tile([P, GMAX * njtot], f32)
            for g in range(GMAX):
                nc.vector.tensor_tensor(
                    scr2[:, g * njtot:(g + 1) * njtot],
                    wg_t[:, g * njtot:(g + 1) * njtot], val[:, :], op=OP.mult)
            csum = sb.tile([P, GMAX], f32)
            nc.vector.tensor_reduce(
                csum[:, :],
                scr2[:, :].rearrange("p (g j) -> p g j", g=GMAX),
                AX.X, OP.add)
            avg = sb.tile([P, GMAX], f32)
            nc.vector.tensor_tensor(avg[:, :], csum[:, :], rcnt_t, op=OP.mult)

            # --- late activations ---
            ce = sb.tile([P, njtot], f32)
            nc.scalar.activation(ce[:, :], val[:, :], AF.Exp)
            ae = sb.tile([P, GMAX], f32)
            nc.scalar.activation(ae[:, :], avg[:, :], AF.Exp, scale=-1.0)
            spl = sb.tile([P, njtot], f32)
            nc.scalar.activation(spl[:, :], ce[:, :], AF.Ln, bias=1.0)
            t1c = sb.tile([P, 1], f32)
            t1 = sb.tile([P, GMAX], f32)
            nc.scalar.activation(t1[:, :], ae[:, :], AF.Ln, bias=1.0,
                                 accum_out=t1c[:, :])

            # --- combine ---
            corr = sb.tile([P, 1], f32)
            scr3 = sb.tile([P, njtot], f32)
            nc.vector.tensor_tensor(scr3[:, :], wcorr_t, spl[:, :], op=OP.mult)
            nc.vector.tensor_reduce(corr[:, :], scr3[:, :], AX.X, OP.add)

            total = sb.tile([P, 1], f32)
            nc.vector.tensor_tensor(total[:, :], t1c[:, :], corr[:, :],
                                    op=OP.add)
            stot = sb.tile([P, 1], f32)
            nc.vector.tensor_tensor(stot[:, :], sacc[0][:, :], sacc[1][:, :],
                                    op=OP.add)
            sacc2 = sb.tile([P, 1], f32)
            nc.vector.tensor_scalar_mul(sacc2[:, :], stot[:, :], TSCALE)
            nc.vector.tensor_tensor(total[:, :], total[:, :], sacc2[:, :],
                                    op=OP.add)
            nc.vector.tensor_tensor(total[:HP, :], total[:HP, :], hacc[:, :],
                                    op=OP.add)

            gtot = sb.tile([P, 1], f32)
            nc.gpsimd.partition_all_reduce(gtot[:, :], total[:, :],
                                           channels=P,
                                           reduce_op=bass_isa.ReduceOp.add)
            res = sb.tile([1, 1], f32)
            nc.vector.tensor_scalar_mul(res[:, :], gtot[0:1, :], 1.0 / B)
            nc.sync.dma_start(out=out[:, :], in_=res[:, :])

    with _act_table_patch():
        nc.compile()
    return nc


def get_graph(meta, enable_asserts=False):
    key = (meta, enable_asserts)
    if key not in _CACHE:
        _CACHE[key] = _build(meta, enable_asserts=enable_asserts)
    return _CACHE[key]


def run(logits, candidates, sampled_indices, trace=False, **kw):
    """Returns (scalar float32 loss, BassKernelResults)."""
    from concourse.bass_utils import run_bass_kernel_spmd

    in_maps, meta = prep_inputs(logits, candidates, sampled_indices)
    nc = get_graph(meta)
    res = run_bass_kernel_spmd(nc, in_maps, core_ids=list(range(NCORES)),
                               trace=trace, **kw)
    partials = [r["out"].reshape(()) for r in res.results]
    loss = np.float32(np.sum(np.stack(partials), dtype=np.float64))
    return loss, res


def kernel(logits, candidates, sampled_indices):
    loss, _ = run(logits, candidates, sampled_indices, trace=False)
    return loss


# revision 16
# speedup vs baseline: 1.3910x; 1.1575x over previous
"""AdaptiveCLPL loss on 8 TRN2 NeuronCores (Bass/Tile) — v3.

loss = mean_b [ psi(avg_cand) + sum_head psi(-l)*(1-mask) + ts*sum_samp psi(-l)*(1-is_cand) ]
with psi(u) = softplus(-u) = Ln(Exp(-u)+1) (no native softplus table).

Decomposition (only term1 is per-row nonlinear; everything else sums):
  total = sum_b softplus(-avg_b)
        + [sum_{head block} softplus(l)    - sum_k uniq*inhead*softplus(l_cand)]
        + ts*[sum_{sampled rows} softplus(l) - sum_k uniq*mult*softplus(l_cand)]

Per-core layout: transposed batch shard lT = logits[rows_perm].T ([C, RB]
row-major); every lT row is a 1KB chunk addressed by class. Candidate values
come from dma_gather (one 1KB descriptor per candidate). Key points:
  - overlapping int16 windows [0,32768) and [C-32768, C): candidates in the
    overlap go to either window, so every partition holds EXACTLY nj0+nj1
    candidate slots -> zero descriptor padding (2560 descriptors, the floor).
  - sampled rows ride the same two gather calls as extra trailing indices
    (num_idxs = 1280+ns_w), replacing the slow gpsimd indirect DMA. Each
    gather call has ~1.4us fixed cost, so only 2 real calls are used.
  - a 16-idx dummy gather issued first pays the gpsimd 'mlp' library IRAM
    load while the idx/aux DMAs are in flight; the 2MB head DMA is gated
    behind it so the library image isn't bandwidth-starved.
  - act tables are doctored at compile time so Exp and Ln resolve to the one
    table set that contains both -> a single ACT_TABLE_LOAD, no swaps.
  - rows are packed 2 per partition; the shard column of row (p,g) is 2p+g.
"""

import numpy as np

B, C, K = 2048, 50000, 10
HEAD, S = 2000, 100
TSCALE = float(C - HEAD) / float(S)  # 480.0
NCORES = 8
RB = B // NCORES  # 256 rows per core
P = 128
HP = 125          # head tile partitions; 2000 rows = 125 * 16
HB = HEAD // HP   # 16 blocks -> 16KB contiguous per partition
ES = 256          # chunk = one lT row (1KB)
WIN = 32768
LO1 = C - WIN     # 17232; window1 = [LO1, C)
GMAX = 2          # exactly 2 rows per partition

_CACHE = {}


def _pack_rows(h0, h1, nj_target, rng):
    """Pair 2*P rows into P partitions s.t. per-partition hard-window counts
    stay <= nj_target. Returns part[r] in [0,P)."""
    nrows = len(h0)
    order = np.argsort(-h0, kind="stable")
    part = np.zeros(nrows, np.int64)
    for i in range(P):
        part[order[i]] = i
        part[order[nrows - 1 - i]] = i
    H0 = np.bincount(part, weights=h0, minlength=P)
    H1 = np.bincount(part, weights=h1, minlength=P)

    def viol(a0, a1):
        return max(a0 - nj_target, 0) + max(a1 - nj_target, 0)

    cur = sum(viol(H0[p], H1[p]) for p in range(P))
    it = 0
    while cur > 0 and it < 20000:
        it += 1
        a, b = rng.integers(0, nrows, 2)
        pa, pb = part[a], part[b]
        if pa == pb:
            continue
        old = viol(H0[pa], H1[pa]) + viol(H0[pb], H1[pb])
        H0[pa] += h0[b] - h0[a]; H1[pa] += h1[b] - h1[a]
        H0[pb] += h0[a] - h0[b]; H1[pb] += h1[a] - h1[b]
        new = viol(H0[pa], H1[pa]) + viol(H0[pb], H1[pb])
        if new <= old:
            part[a], part[b] = pb, pa
            cur += new - old
        else:
            H0[pa] -= h0[b] - h0[a]; H1[pa] -= h1[b] - h1[a]
            H0[pb] -= h0[a] - h0[b]; H1[pb] -= h1[a] - h1[b]
    return part, cur == 0


def prep_inputs(logits, candidates, sampled_indices):
    """Full inputs -> (in_maps, meta). Host work is sharding + index math only."""
    logits = np.asarray(logits)
    candidates = np.asarray(candidates)
    sampled_indices = np.asarray(sampled_indices)
    assert logits.shape == (B, C) and candidates.shape == (B, K)
    srow = (HEAD + sampled_indices.astype(np.int64)).astype(np.int64)  # [S]
    svals, scounts = np.unique(srow, return_counts=True)
    smult = dict(zip(svals.tolist(), scounts.tolist()))

    # sampled rows -> windows (balance the flexible ones)
    s_w = np.where(srow < LO1, 0, np.where(srow >= WIN, 1, -1))
    flex = np.where(s_w < 0)[0]
    n0 = int((s_w == 0).sum())
    n1 = int((s_w == 1).sum())
    for j in flex:
        if n0 <= n1:
            s_w[j] = 0; n0 += 1
        else:
            s_w[j] = 1; n1 += 1
    ns0, ns1 = n0, n1
    sidx_w = [srow[s_w == 0] - 0, srow[s_w == 1] - LO1]

    rng = np.random.default_rng(12345)
    cores = []
    nj_need = [1, 1]
    for i in range(NCORES):
        rows = slice(i * RB, (i + 1) * RB)
        cand = candidates[rows].astype(np.int64)          # [RB, K]
        valid = cand >= 0
        uniq = valid.copy()
        for k in range(1, K):
            dup = (cand[:, :k] == cand[:, k:k + 1]).any(axis=1)
            uniq[:, k] &= ~dup
        uniqf = uniq.astype(np.float32)
        cnt = np.maximum((uniq & valid).sum(axis=1), 1).astype(np.float32)
        inhead = (cand < HEAD).astype(np.float32)
        mult = np.vectorize(lambda c: smult.get(int(c), 0))(cand).astype(np.float32)
        wcorr_rk = -uniqf * (inhead + TSCALE * mult)      # [RB, K]

        h0 = (valid & (cand < LO1)).sum(axis=1)
        h1 = (valid & (cand >= WIN)).sum(axis=1)
        part, ok = _pack_rows(h0.astype(np.int64), h1.astype(np.int64), K, rng)
        grp = np.zeros(RB, np.int64)
        seen = {}
        for r in range(RB):
            p = int(part[r])
            grp[r] = seen.get(p, 0)
            seen[p] = grp[r] + 1
        assert max(seen.values()) <= GMAX

        # window assignment per candidate
        cw = np.full((RB, K), -1, np.int64)
        cw[valid & (cand < LO1)] = 0
        cw[valid & (cand >= WIN)] = 1
        H0 = np.bincount(part, weights=(cw == 0).sum(1), minlength=P).astype(np.int64)
        for r in range(RB):
            p = int(part[r])
            for k in range(K):
                if valid[r, k] and cw[r, k] < 0:
                    if H0[p] < K:
                        cw[r, k] = 0; H0[p] += 1
                    else:
                        cw[r, k] = 1
        W0c = np.bincount(part, weights=(cw == 0).sum(1), minlength=P).astype(np.int64)
        W1c = np.bincount(part, weights=(cw == 1).sum(1), minlength=P).astype(np.int64)
        nj_need[0] = max(nj_need[0], int(W0c.max()))
        nj_need[1] = max(nj_need[1], int(W1c.max()))
        cores.append((cand, valid, uniqf, cnt, wcorr_rk, part, grp, cw))

    nj0, nj1 = nj_need
    njtot = nj0 + nj1
    meta = (nj0, nj1, ns0, ns1)

    # idx16 layout: [w0 cand+samp | w1 cand+samp] wrapped per call
    ni = [nj0 * P + ns0, nj1 * P + ns1]          # num_idxs per call
    ncols = [(-(-n // 16)) for n in ni]
    AUXW = njtot + njtot + GMAX * njtot + GMAX + ES

    in_maps = []
    for i in range(NCORES):
        cand, valid, uniqf, cnt, wcorr_rk, part, grp, cw = cores[i]
        rows = slice(i * RB, (i + 1) * RB)
        col = (2 * part + grp).astype(np.int64)
        perm = np.zeros(RB, np.int64)
        perm[col] = np.arange(RB)
        lT = np.ascontiguousarray(logits[rows][perm].T.astype(np.float32, copy=False))
        # head rows reshaped to 128 partitions so the HWDGE DMA spreads
        # across all 16 SDMA engines (a [125, *] shape lands on only 5)
        lH = np.ascontiguousarray(lT[:HEAD].reshape(P, HEAD * RB // P))

        idxs = [np.zeros(nj0 * P, np.int16), np.zeros(nj1 * P, np.int16)]
        offt = np.full((P, njtot), -1.0, np.float32)
        wcorr = np.zeros((P, njtot), np.float32)
        wg = np.zeros((P, GMAX * njtot), np.float32)
        fill = np.zeros((P, 2), np.int64)
        base_j = [0, nj0]
        base_lo = [0, LO1]
        for r in range(RB):
            p, g = int(part[r]), int(grp[r])
            for k in range(K):
                if not valid[r, k]:
                    continue
                w = int(cw[r, k])
                j = int(fill[p, w]); fill[p, w] += 1
                idxs[w][j * P + p] = cand[r, k] - base_lo[w]
                jj = base_j[w] + j
                offt[p, jj] = float(col[r])
                wcorr[p, jj] = wcorr_rk[r, k]
                wg[p, g * njtot + jj] = uniqf[r, k]
        rcnt = np.zeros((P, GMAX), np.float32)
        rcnt[part, grp] = 1.0 / cnt
        iota = np.broadcast_to(np.arange(ES, dtype=np.float32), (P, ES)).copy()

        cols_out = []
        for w in (0, 1):
            flat = np.zeros(ncols[w] * 16, np.int16)
            flat[:len(idxs[w])] = idxs[w]
            flat[len(idxs[w]):len(idxs[w]) + len(sidx_w[w])] = (
                sidx_w[w].astype(np.int16))
            wrapped = flat.reshape(ncols[w], 16).T
            cols_out.append(np.tile(wrapped, (8, 1)))
        idx16 = np.ascontiguousarray(np.concatenate(cols_out, axis=1))
        assert idx16.shape == (P, sum(ncols))

        auxcat = np.ascontiguousarray(np.concatenate(
            [offt, wcorr, wg, rcnt, iota], axis=1))
        assert auxcat.shape == (P, AUXW)
        in_maps.append({"lT": lT, "lH": lH, "idx16": idx16, "aux": auxcat})
    return in_maps, meta


def _act_table_patch():
    """Context manager: make Exp and Ln resolve only to the one act-func set
    ('natural_log_exp_and_others') that holds both, so the compile-time table
    placement emits a single ACT_TABLE_LOAD instead of swapping per phase."""
    import contextlib
    from concourse import hw_specs, mybir

    import contextlib as _ctl
    from concourse import bacc as _bacc

    @_ctl.contextmanager
    def ctx():
        real = hw_specs.get_activation_tables
        AF = mybir.ActivationFunctionType

        def doctored(arch):
            tabs = {k: set(v) for k, v in real(arch).items()}
            if any(AF.Exp in v and AF.Ln in v for v in tabs.values()):
                for name, s in tabs.items():
                    if not (AF.Exp in s and AF.Ln in s):
                        s.discard(AF.Exp)
                        s.discard(AF.Ln)
            return tabs

        hw_specs.get_activation_tables = doctored
        _bacc.get_activation_tables = doctored
        try:
            yield
        finally:
            hw_specs.get_activation_tables = real
            _bacc.get_activation_tables = real

    return ctx()


def _build(meta, enable_asserts=False):
    import concourse.bass as bass
    import concourse.tile as tile
    from concourse import bacc, bass_isa, mybir
    from concourse.bass import _add_dep_helper

    nj0, nj1, ns0, ns1 = meta
    njtot = nj0 + nj1
    njs = [nj0, nj1]
    nss = [ns0, ns1]
    ni = [nj0 * P + ns0, nj1 * P + ns1]
    ncols = [(-(-n // 16)) for n in ni]
    AUXW = njtot + njtot + GMAX * njtot + GMAX + ES

    f32 = mybir.dt.float32
    i16 = mybir.dt.int16
    AF = mybir.ActivationFunctionType
    OP = mybir.AluOpType
    AX = mybir.AxisListType

    nc = bacc.Bacc(
        "TRN2",
        target_bir_lowering=False,
        debug=False,
        enable_asserts=enable_asserts,
        num_devices=NCORES,
        num_swdge_queues=2,
    )

    HW_ = HEAD * RB // P  # 4000
    lT = nc.dram_tensor("lT", [C, RB], f32, kind="ExternalInput").ap()
    lH = nc.dram_tensor("lH", [P, HW_], f32, kind="ExternalInput").ap()
    idx16 = nc.dram_tensor("idx16", [P, sum(ncols)], i16,
                           kind="ExternalInput").ap()
    aux = nc.dram_tensor("aux", [P, AUXW], f32, kind="ExternalInput").ap()
    out = nc.dram_tensor("out", [1, 1], f32, kind="ExternalOutput").ap()

    with tile.TileContext(nc) as tc:
        with tc.tile_pool(name="sb", bufs=1) as sb:
            # --- tiles ---
            dummy_idx = sb.tile([P, 1], i16)
            gdummy = sb.tile([P, ES], f32)
            idx16_t = sb.tile([P, sum(ncols)], i16)
            aux_t = sb.tile([P, AUXW], f32)
            gwin = [sb.tile([P, (njs[w] + 1) * ES], f32, name=f"gwin{w}")
                    for w in (0, 1)]
            ht = sb.tile([P, HW_], f32)
            msk = sb.tile([P, njtot * ES], f32)
            val = sb.tile([P, njtot], f32)

            # --- early memsets (vector) + small input DMAs (scalar ring) ---
            nc.vector.memset(dummy_idx[:, :], 0)
            nc.vector.memset(gwin[0][:, njs[0] * ES:], -50.0)
            nc.vector.memset(gwin[1][:, njs[1] * ES:], -50.0)
            nc.scalar.dma_start(out=idx16_t[:, :], in_=idx16[:, :])
            nc.scalar.dma_start(out=aux_t[:, :], in_=aux[:, :])

            # prime the act table early: the single Exp+Ln table set loads
            # behind this no-dep activation instead of before the first
            # real one at ~50us
            prime = sb.tile([1, 1], f32)
            nc.vector.memset(prime[:, :], 0.0)
            nc.scalar.activation(prime[:, :], prime[:, :], AF.Exp, scale=0.0)

            o = 0
            offt_t = aux_t[:, o:o + njtot]; o += njtot
            wcorr_t = aux_t[:, o:o + njtot]; o += njtot
            wg_t = aux_t[:, o:o + GMAX * njtot]; o += GMAX * njtot
            rcnt_t = aux_t[:, o:o + GMAX]; o += GMAX
            iota_t = aux_t[:, o:o + ES]; o += ES

            # --- gathers on gpsimd (dummy first: pays the mlp IRAM load) ---
            gdum = nc.gpsimd.dma_gather(
                out_ap=gdummy[:, :].rearrange("p (j e) -> p j e", e=ES),
                in_ap=lT[0:16, :], idxs_ap=dummy_idx[:, :],
                num_idxs=16, num_idxs_reg=16, elem_size=ES,
                single_packet=False)

            oc = 0
            gcalls = []
            for w in (0, 1):
                lo = 0 if w == 0 else LO1
                g = nc.gpsimd.dma_gather(
                    out_ap=gwin[w][:, :].rearrange("p (j e) -> p j e", e=ES),
                    in_ap=lT[lo:lo + WIN, :],
                    idxs_ap=idx16_t[:, oc:oc + ncols[w]],
                    num_idxs=ni[w], num_idxs_reg=ni[w], elem_size=ES,
                    single_packet=False, queue_num=w)
                gcalls.append(g)
                oc += ncols[w]

            # --- head DMA on sync ring, gated behind the library load so the
            # ucode image isn't bandwidth-starved ---
            d_h0 = nc.sync.dma_start(out=ht[:, :], in_=lH[:, :])
            _add_dep_helper(d_h0.ins, gdum.ins, sync=True,
                            reason="head DMA after gpsimd lib load")

            # --- vector: eq masks early, then per-window extract ---
            nc.vector.tensor_tensor(
                out=msk[:, :].rearrange("p (j e) -> p j e", e=ES),
                in0=iota_t.unsqueeze(1).to_broadcast([P, njtot, ES]),
                in1=offt_t.unsqueeze(2).to_broadcast([P, njtot, ES]),
                op=OP.is_equal)
            jo = 0
            for w in (0, 1):
                nc.vector.tensor_tensor(
                    msk[:, jo * ES:(jo + njs[w]) * ES],
                    msk[:, jo * ES:(jo + njs[w]) * ES],
                    gwin[w][:, :njs[w] * ES], op=OP.mult)
                nc.vector.tensor_reduce(
                    val[:, jo:jo + njs[w]],
                    msk[:, jo * ES:(jo + njs[w]) * ES].rearrange(
                        "p (j e) -> p j e", e=ES),
                    AX.X, OP.add)
                jo += njs[w]

            # --- scalar: softplus everywhere (single Exp+Ln table set) ---
            sacc = [sb.tile([P, 1], f32, name=f"sacc{w}") for w in (0, 1)]
            e_s = [nc.scalar.activation(gwin[w][:, njs[w] * ES:],
                                        gwin[w][:, njs[w] * ES:], AF.Exp)
                   for w in (0, 1)]
            l_s = [nc.scalar.activation(gwin[w][:, njs[w] * ES:],
                                        gwin[w][:, njs[w] * ES:], AF.Ln,
                                        bias=1.0, accum_out=sacc[w][:, :])
                   for w in (0, 1)]
            hacc = sb.tile([P, 1], f32)
            nc.scalar.activation(ht[:, :], ht[:, :], AF.Exp)
            nc.scalar.activation(ht[:, :], ht[:, :], AF.Ln, bias=1.0,
                                 accum_out=hacc[:, :])

            # --- term1: per-row avg over candidates ---
            scr2 = sb.tile([P, GMAX * njtot], f32)
            for g in range(GMAX):
                nc.vector.tensor_tensor(
                    scr2[:, g * njtot:(g + 1) * njtot],
                    wg_t[:, g * njtot:(g + 1) * njtot], val[:, :], op=OP.mult)
            csum = sb.tile([P, GMAX], f32)
            nc.vector.tensor_reduce(
                csum[:, :],
                scr2[:, :].rearrange("p (g j) -> p g j", g=GMAX),
                AX.X, OP.add)
            avg = sb.tile([P, GMAX], f32)
            nc.vector.tensor_tensor(avg[:, :], csum[:, :], rcnt_t, op=OP.mult)

            # --- late activations ---
            ce = sb.tile([P, njtot], f32)
            nc.scalar.activation(ce[:, :], val[:, :], AF.Exp)
            ae = sb.tile([P, GMAX], f32)
            nc.scalar.activation(ae[:, :], avg[:, :], AF.Exp, scale=-1.0)
            spl = sb.tile([P, njtot], f32)
            nc.scalar.activation(spl[:, :], ce[:, :], AF.Ln, bias=1.0)
            t1c = sb.tile([P, 1], f32)
            t1 = sb.tile([P, GMAX], f32)
            nc.scalar.activation(t1[:, :], ae[:, :], AF.Ln, bias=1.0,
                                 accum_out=t1c[:, :])

            # --- combine ---
            corr = sb.tile([P, 1], f32)
            scr3 = sb.tile([P, njtot], f32)
            nc.vector.tensor_tensor(scr3[:, :], wcorr_t, spl[:, :], op=OP.mult)
            nc.vector.tensor_reduce(corr[:, :], scr3[:, :], AX.X, OP.add)

            total = sb.tile([P, 1], f32)
            nc.vector.tensor_tensor(total[:, :], t1c[:, :], corr[:, :],
                                    op=OP.add)
            stot = sb.tile([P, 1], f32)
            nc.vector.tensor_tensor(stot[:, :], sacc[0][:, :], sacc[1][:, :],
                                    op=OP.add)
            sacc2 = sb.tile([P, 1], f32)
            nc.vector.tensor_scalar_mul(sacc2[:, :], stot[:, :], TSCALE)
            nc.vector.tensor_tensor(total[:, :], total[:, :], sacc2[:, :],
                                    op=OP.add)
            nc.vector.tensor_tensor(total[:, :], total[:, :], hacc[:, :],
                                    op=OP.add)

            gtot = sb.tile([P, 1], f32)
            nc.gpsimd.partition_all_reduce(gtot[:, :], total[:, :],
                                           channels=P,
                                           reduce_op=bass_isa.ReduceOp.add)
            res = sb.tile([1, 1], f32)
            nc.vector.tensor_scalar_mul(res[:, :], gtot[0:1, :], 1.0 / B)
            nc.sync.dma_start(out=out[:, :], in_=res[:, :])

    with _act_table_patch():
        nc.compile()
    return nc


def get_graph(meta, enable_asserts=False):
    key = (meta, enable_asserts)
    if key not in _CACHE:
        _CACHE[key] = _build(meta, enable_asserts=enable_asserts)
    return _CACHE[key]


def run(logits, candidates, sampled_indices, trace=False, **kw):
    """Returns (scalar float32 loss, BassKernelResults)."""
    from concourse.bass_utils import run_bass_kernel_spmd

    in_maps, meta = prep_inputs(logits, candidates, sampled_indices)
    nc = get_graph(meta)
    res = run_bass_kernel_spmd(nc, in_maps, core_ids=list(range(NCORES)),
                               trace=trace, **kw)
    partials = [r["out"].reshape(()) for r in res.results]
    loss = np.float32(np.sum(np.stack(partials), dtype=np.float64))
    return loss, res


def kernel(logits, candidates, sampled_indices):
    loss, _ = run(logits, candidates, sampled_indices, trace=False)
    return loss
